# revision 41
# baseline (speedup 1.0000x reference)
"""CFConv (SchNet continuous-filter convolution) on 8 TRN2 NeuronCores.

    h   = softplus(rbf @ w1 + b1)        # [N, NB, F]
    W   = h @ w2 + b2                    # [N, NB, F]
    out = sum_n x[neighbors] * W         # [N, F]

Sharding: atoms (dim 0) split 8 ways; x + filter weights replicated.
No collectives needed - each core gathers its neighbors from its own
full copy of x in DRAM.

Per-core dataflow (feature-major [f, pair] layout for the filter math):
  SWDGE:      xjg[p, g, e] = x[nbr[g*128+p], e]   (dma_gather
              transpose=False, split across all 4 SWDGE queues so the
              four Q7 core pairs generate descriptors concurrently;
              transpose=True mode is single-queue-only: concurrent
              transposed streams corrupt each other in the shared xbar)
  mm1 (PE):   h_pre[f, p] = w1[r, f].T @ rbf_t[r, p]      -> PSUM f32
  ACT:        e = exp(h_pre + b1)  (b1 = per-partition bias AP)
  ACT:        h = softplus = ln(e + 1)  (exp+ln pinned to the one
              act-func table containing both -> no table reloads)
  mm2 (PE):   W[f, p] = w2[g, f].T @ h[g, p]              -> PSUM f32
  PE:         xjt[f, p] = transpose(xjg group)            -> PSUM f32
  DVE STT:    prod = (W + b2) * xjt   (both operands read from PSUM)
  DVE:        out[f, a] = tensor_reduce over the 32-neighbor axis
Output is [128 f, 2500 atoms] f32 per core; host transposes + concats.
"""

import os

import numpy as np

import concourse.bass as bass
import concourse.bacc as bacc
import concourse.mybir as mybir
import concourse.tile as tile
from contextlib import ExitStack

N_ATOMS = 20000
NB = 32
F = 128
R = 64
NCORES = 8
NA = N_ATOMS // NCORES          # atoms per core     = 2500
NP = NA * NB                    # pairs per core     = 80000
CH = 1024                       # pairs per chunk
SPAN = 8                        # chunks per span (gather/reduce granularity)
NQ = 4                          # SWDGE queues (Q7 core pairs) for gathers

f16 = mybir.dt.float16
f32 = mybir.dt.float32
i16 = mybir.dt.int16

_CACHE = {}


class _Bacc(bacc.Bacc):
    """Bacc with exp+ln pinned to the one act-func table containing both.

    The stock placement pass picks the first act_info.json set containing
    each activation function: Exp -> "exp_and_others", Ln -> "natural_log",
    which alternates ACT_TABLE_LOADs (~1.5us each) on every exp/ln switch.
    Removing Exp/Ln from every other set (list order preserved, so the
    positional act_func_set_id stays valid) pins both to
    "natural_log_exp_and_others" -> one load for the whole kernel.
    """

    def insert_act_table_loads(self):
        from concourse.hw_specs import get_activation_tables
        from concourse.bacc import _bass_rust

        has_activation = any(
            isinstance(i, mybir.InstActivation)
            for b in self.main_func.blocks
            for i in b.instructions
        )
        if not has_activation:
            return
        tables = list(get_activation_tables(self.m.arch).items())
        shared = {mybir.ActivationFunctionType.Exp, mybir.ActivationFunctionType.Ln}
        tables = [
            (name, s if name == "natural_log_exp_and_others" else (s - shared))
            for name, s in tables
        ]
        _bass_rust.insert_act_table_loads(self, tables)


def _chunks():
    """(offset, width) pairs covering [0, NP), width % 128 == 0."""
    out = []
    off = 0
    while off < NP:
        w = min(CH, NP - off)
        out.append((off, w))
        off += w
    return out


def _spans():
    """Group chunks into spans of up to SPAN chunks."""
    ch = _chunks()
    spans = []
    for i in range(0, len(ch), SPAN):
        group = ch[i : i + SPAN]
        s0 = group[0][0]
        sp = sum(w for _, w in group)
        spans.append((s0, sp, group))
    return spans


def _qsplit(sp):
    """Split sp into NQ (offset, width) parts, each width % 128 == 0.

    Queue 0 runs the transposed gather, whose Q7 descriptor generation is
    ~20% slower per index than the plain mode on queues 1-3 - give it a
    correspondingly smaller share so all four Q7 pairs finish together.
    """
    n128 = sp // 128
    t = max(1, (n128 * 3) // 16)
    parts = [(0, t * 128)]
    rest = n128 - t
    off = t * 128
    for q in range(1, NQ):
        w = (rest // (NQ - 1) + (1 if (q - 1) < rest % (NQ - 1) else 0)) * 128
        if w:
            parts.append((off, w))
        off += w
    return parts


def _build():
    if "nc" in _CACHE:
        return _CACHE["nc"]
    nc = _Bacc(num_swdge_queues=NQ)

    x_d = nc.declare_dram_parameter("x", [N_ATOMS, F], f16, isOutput=False)
    rbf_d = nc.declare_dram_parameter("rbf_t", [R, NP], f16, isOutput=False)
    idx_d = nc.declare_dram_parameter("idx", [128, NP // 16], i16, isOutput=False)
    w1_d = nc.declare_dram_parameter("w1", [R, F], f16, isOutput=False)
    w2_d = nc.declare_dram_parameter("w2", [F, F], f16, isOutput=False)
    b1_d = nc.declare_dram_parameter("b1", [F, 1], f32, isOutput=False)
    b2_d = nc.declare_dram_parameter("b2", [F, 1], f32, isOutput=False)
    id_d = nc.declare_dram_parameter("ident", [F, F], f16, isOutput=False)
    out_d = nc.declare_dram_parameter("out", [F, NA], f32, isOutput=True)

    spans = _spans()
    max_sp = max(sp for _, sp, _ in spans)

    with tile.TileContext(nc) as tc, ExitStack() as ctx:
        consts = ctx.enter_context(tc.tile_pool(name="consts", bufs=1))
        # Span inputs (rbf slab + gathered x) prefetch 2 spans (16K
        # pairs) ahead; span-local scratch rotates 2-deep.
        inpool = ctx.enter_context(tc.tile_pool(name="inpool", bufs=2))
        scpool = ctx.enter_context(tc.tile_pool(name="scpool", bufs=2))
        # PSUM budget (8 banks of 512 f32): pp [128,CH] f32 = 2 banks
        # x3 bufs = 6 (mm1 writes it, exp reads it, then mm2 reuses it -
        # exp -> mm2 is a true dependency anyway, so the reuse costs no
        # ordering); xt (PE-transposed x_j, f16) [128,CH] = 1 bank x2 = 2.
        ppool = ctx.enter_context(tc.tile_pool(name="pp", bufs=3, space="PSUM"))
        xtpool = ctx.enter_context(tc.tile_pool(name="xt", bufs=2, space="PSUM"))

        # consts arrive via the ACT HWDGE queue: the sync queue then only
        # carries the latency-critical per-span rbft loads and output
        # stores, so span 0's rbft is never stuck behind the idx table.
        idxs = consts.tile([128, NP // 16], i16)
        sp0 = _spans()[0][1]
        nc.scalar.dma_start(
            out=idxs[:, : sp0 // 16], in_=idx_d[:, : sp0 // 16]
        )
        w1s = consts.tile([R, F], f16)
        nc.scalar.dma_start(out=w1s, in_=w1_d[:])
        w2s = consts.tile([F, F], f16)
        nc.scalar.dma_start(out=w2s, in_=w2_d[:])
        b1s = consts.tile([F, 1], f32)
        nc.scalar.dma_start(out=b1s, in_=b1_d[:])
        b2s = consts.tile([F, 1], f32)
        nc.scalar.dma_start(out=b2s, in_=b2_d[:])
        ids = consts.tile([F, F], f16)
        nc.scalar.dma_start(out=ids, in_=id_d[:])
        # DVE-local copy of b2: the STT multiply runs on DVE, so reading a
        # DVE-written tile adds no cross-engine sync wait.
        b2v = consts.tile([F, 1], f32)
        nc.vector.tensor_copy(out=b2v, in_=b2s)
        # (idx slice for span 0 was issued first above, so the first
        # gathers only wait on 64KB; the rest of the table follows here.)
        nc.scalar.dma_start(
            out=idxs[:, sp0 // 16 :], in_=idx_d[:, sp0 // 16 :]
        )
        outst = consts.tile([F, NA], f32)

        for s0, sp, group in spans:
            atom0 = s0 // NB
            natoms = sp // NB

            rbft = inpool.tile([R, max_sp], f16, tag="rbft")
            nc.sync.dma_start(out=rbft[:, :sp], in_=rbf_d[:, s0 : s0 + sp])

            xjg = inpool.tile([128, max_sp], f16, tag="xjg")
            xjt = inpool.tile([128, max_sp], f16, tag="xjt")
            xv = xjg.rearrange("p (g e) -> p g e", e=F)
            # Hybrid 4-way gather, one SWDGE queue per Q7 core pair:
            #  - queue 0 runs the TRANSPOSED gather (sole user of the
            #    SDMA xbar - concurrent transposed streams corrupt each
            #    other) and lands feature-major directly in xjt[:, :t].
            #  - queues 1-3 gather non-transposed into xjg; those groups
            #    are PE-transposed into xjt later.
            qparts = _qsplit(sp)
            t0w = qparts[0][1]
            nc.gpsimd.dma_gather(
                xjt[:, :t0w].rearrange("p (o n) -> p o n", o=1),
                x_d[:],
                idxs[:, s0 // 16 : (s0 + t0w) // 16],
                t0w,
                t0w,
                F,
                transpose=True,
                # single_packet=True crashes the SDMA engine above ~512
                # indices (NRT_EXEC_UNIT_UNRECOVERABLE) - packets are
                # capped at 64 descriptors.
                single_packet=False,
                queue_num=0,
            )
            for q, (qoff, qw) in enumerate(qparts[1:], start=1):
                nc.gpsimd.dma_gather(
                    xv[:, (qoff - t0w) // F : (qoff - t0w + qw) // F, :],
                    x_d[:],
                    idxs[:, (s0 + qoff) // 16 : (s0 + qoff + qw) // 16],
                    qw,
                    qw,
                    F,
                    transpose=False,
                    single_packet=False,
                    queue_num=q,
                )

            # es and prod share one tile: exp writes it, ln consumes it,
            # then the STT overwrites the same columns (ordered by the
            # exp->ln->mm2->STT dependency chain) and the reduce reads it.
            prod = scpool.tile([128, max_sp], f16, tag="eprod")
            es = prod
            hsp = scpool.tile([128, max_sp], f16, tag="hsp")

            # Chunk loop, software-pipelined by one stage: mm2/STT of
            # chunk c-1 issue after mm1/transposes of chunk c, so the PE
            # never head-of-line blocks on ACT's ln while a ready mm1
            # waits behind mm2.
            def stage1(ci, off, w):
                co = off - s0
                pp = ppool.tile([128, CH], f32)
                for o in range(0, w, 512):
                    n = min(512, w - o)
                    nc.tensor.matmul(
                        pp[:, o : o + n],
                        w1s[:],
                        rbft[:, co + o : co + o + n],
                        start=True,
                        stop=True,
                    )
                # e = exp(h_pre + b1), then h = softplus = ln(e + 1); both
                # resolve to the natural_log_exp_and_others table -> no
                # table switches.
                nc.scalar.activation(
                    out=es[:, co : co + w],
                    in_=pp[:, :w],
                    func=mybir.ActivationFunctionType.Exp,
                    bias=b1s[:, 0:1],
                    scale=1.0,
                )
                nc.scalar.activation(
                    out=hsp[:, co : co + w],
                    in_=es[:, co : co + w],
                    func=mybir.ActivationFunctionType.Ln,
                    bias=1.0,
                    scale=1.0,
                )
                # xjt[f, p] per 128-pair group via PE transpose
                # (SBUF->PSUM), for the groups queue 0 didn't already land
                # feature-major.
                tco = max(co, t0w)
                if tco < co + w:
                    xt = xtpool.tile([128, CH], f16)
                    for g in range(tco - co, w, F):
                        nc.tensor.transpose(
                            xt[:, g : g + F],
                            xv[:, (co + g - t0w) // F, :],
                            ids[:],
                        )
                    # DVE reads only ONE non-scalar operand from PSUM; stage
                    # the transposed x_j in SBUF first. Alternate the copy
                    # between DVE and ACT to balance engine load.
                    if ci % 2 == 0:
                        nc.vector.tensor_copy(
                            out=xjt[:, tco : co + w],
                            in_=xt[:, tco - co : w],
                        )
                    else:
                        nc.scalar.activation(
                            out=xjt[:, tco : co + w],
                            in_=xt[:, tco - co : w],
                            func=mybir.ActivationFunctionType.Copy,
                            bias=0.0,
                            scale=1.0,
                        )
                return pp

            def stage2(off, w, pp):
                co = off - s0
                for o in range(0, w, 512):
                    n = min(512, w - o)
                    nc.tensor.matmul(
                        pp[:, o : o + n],
                        w2s[:],
                        hsp[:, co + o : co + o + n],
                        start=True,
                        stop=True,
                    )
                # prod = (W + b2) * xjt  (fused bias + multiply, PSUM read)
                nc.vector.scalar_tensor_tensor(
                    out=prod[:, co : co + w],
                    in0=pp[:, :w],
                    scalar=b2v[:, 0:1],
                    in1=xjt[:, co : co + w],
                    op0=mybir.AluOpType.add,
                    op1=mybir.AluOpType.mult,
                )

            def do_reduce(co, wid):
                # neighbor reduction straight into the f32 output staging
                # tile (tensor_reduce is hardwired PERF_ONE, so an f16
                # output would not run any faster). 2-chunk granularity
                # halves the per-instruction and semaphore overhead.
                nc.vector.tensor_reduce(
                    out=outst[:, (s0 + co) // NB : (s0 + co + wid) // NB],
                    in_=prod[:, co : co + wid].rearrange(
                        "p (a n) -> p a n", n=NB
                    ),
                    axis=mybir.AxisListType.X,
                    op=mybir.AluOpType.add,
                )

            pending = None
            red0 = 0
            for ci, (off, w) in enumerate(group):
                pp = stage1(ci, off, w)
                if pending is not None:
                    stage2(*pending)
                    if ci % 2 == 0:
                        # pending (odd ci-1) completed a 2-chunk pair
                        co_end = pending[0] - s0 + pending[1]
                        do_reduce(red0, co_end - red0)
                        red0 = co_end
                pending = (off, w, pp)
            stage2(*pending)
            do_reduce(red0, sp - red0)

            # stream this span's output slab out now (drain overlap)
            nc.sync.dma_start(
                out=out_d[:, atom0 : atom0 + natoms],
                in_=outst[:, atom0 : atom0 + natoms],
            )



    # Bacc.finalize() runs the sync-wait legalization (each TRN2 instruction
    # carries at most one wait; extras are split into event-semaphore insts).
    nc.finalize()
    _CACHE["nc"] = nc
    return nc


def _prep_core_inputs(x16, rbf, neighbors, w1_16, w2_16, b1c, b2c, ident, c):
    a0, a1 = c * NA, (c + 1) * NA
    rbf_t = np.ascontiguousarray(rbf[a0:a1].reshape(NP, R).T.astype(np.float16))
    nb = np.ascontiguousarray(neighbors[a0:a1]).reshape(NP).astype(np.int16)
    # dma_gather index layout: element i lives at [i % 16, i // 16],
    # 16-partition block replicated 8x down the partition dim.
    idx16 = np.ascontiguousarray(nb.reshape(NP // 16, 16).T)
    idx = np.tile(idx16, (8, 1))
    return {
        "x": x16,
        "rbf_t": rbf_t,
        "idx": np.ascontiguousarray(idx),
        "w1": w1_16,
        "w2": w2_16,
        "b1": b1c,
        "b2": b2c,
        "ident": ident,
    }


def kernel(x, rbf, neighbors, w1, b1, w2, b2):
    from concourse.bass_utils import run_bass_kernel_spmd

    x = np.asarray(x)
    rbf = np.asarray(rbf)
    neighbors = np.asarray(neighbors)
    w1 = np.asarray(w1)
    b1 = np.asarray(b1)
    w2 = np.asarray(w2)
    b2 = np.asarray(b2)

    nc = _build()

    x16 = x.astype(np.float16)
    w1_16 = np.ascontiguousarray(w1.astype(np.float16))
    w2_16 = w2.astype(np.float16)
    b1c = np.ascontiguousarray(b1.reshape(F, 1).astype(np.float32))
    b2c = np.ascontiguousarray(b2.reshape(F, 1).astype(np.float32))
    ident = np.eye(F, dtype=np.float16)

    in_maps = [
        _prep_core_inputs(x16, rbf, neighbors, w1_16, w2_16, b1c, b2c, ident, c)
        for c in range(NCORES)
    ]

    res = run_bass_kernel_spmd(
        nc,
        in_maps,
        core_ids=list(range(NCORES)),
        trace=bool(int(os.environ.get("CFCONV_TRACE", "0"))),
    )
    _CACHE["last_result"] = res

    out = np.concatenate(
        [res.results[c]["out"].T for c in range(NCORES)], axis=0
    )
    return np.ascontiguousarray(out.astype(np.float32))


# revision 42
# speedup vs baseline: 1.0972x; 1.0972x over previous
"""CFConv (SchNet continuous-filter convolution) on 8 TRN2 NeuronCores.

    h   = softplus(rbf @ w1 + b1)        # [N, NB, F]
    W   = h @ w2 + b2                    # [N, NB, F]
    out = sum_n x[neighbors] * W         # [N, F]

Sharding: atoms (dim 0) split 8 ways; x + filter weights replicated.
No collectives needed - each core gathers its neighbors from its own
full copy of x in DRAM.

Per-core dataflow (feature-major [f, pair] layout for the filter math):
  SWDGE:      xjg[p, g, e] = x[nbr[g*128+p], e]   (dma_gather
              transpose=False, split across all 4 SWDGE queues so the
              four Q7 core pairs generate descriptors concurrently;
              transpose=True mode is single-queue-only: concurrent
              transposed streams corrupt each other in the shared xbar)
  mm1 (PE):   h_pre[f, p] = w1[r, f].T @ rbf_t[r, p]      -> PSUM f32
  ACT:        e = exp(h_pre + b1)  (b1 = per-partition bias AP)
  ACT:        h = softplus = ln(e + 1)  (exp+ln pinned to the one
              act-func table containing both -> no table reloads)
  mm2 (PE):   W[f, p] = w2[g, f].T @ h[g, p]              -> PSUM f32
  PE:         xjt[f, p] = transpose(xjg group)            -> PSUM f32
  DVE STT:    prod = (W + b2) * xjt   (both operands read from PSUM)
  DVE:        out[f, a] = tensor_reduce over the 32-neighbor axis
Output is [128 f, 2500 atoms] f32 per core; host transposes + concats.
"""

import os

import numpy as np

import concourse.bass as bass
import concourse.bacc as bacc
import concourse.mybir as mybir
import concourse.tile as tile
from contextlib import ExitStack

N_ATOMS = 20000
NB = 32
F = 128
R = 64
NCORES = 8
NA = N_ATOMS // NCORES          # atoms per core     = 2500
NP = NA * NB                    # pairs per core     = 80000
CH = 1024                       # pairs per chunk
SPAN = 4                        # chunks per span (gather/reduce granularity)
NQ = 4                          # SWDGE queues (Q7 core pairs) for gathers

f16 = mybir.dt.float16
f32 = mybir.dt.float32
i16 = mybir.dt.int16

_CACHE = {}


class _Bacc(bacc.Bacc):
    """Bacc with exp+ln pinned to the one act-func table containing both.

    The stock placement pass picks the first act_info.json set containing
    each activation function: Exp -> "exp_and_others", Ln -> "natural_log",
    which alternates ACT_TABLE_LOADs (~1.5us each) on every exp/ln switch.
    Removing Exp/Ln from every other set (list order preserved, so the
    positional act_func_set_id stays valid) pins both to
    "natural_log_exp_and_others" -> one load for the whole kernel.
    """

    def insert_act_table_loads(self):
        from concourse.hw_specs import get_activation_tables
        from concourse.bacc import _bass_rust

        has_activation = any(
            isinstance(i, mybir.InstActivation)
            for b in self.main_func.blocks
            for i in b.instructions
        )
        if not has_activation:
            return
        tables = list(get_activation_tables(self.m.arch).items())
        shared = {mybir.ActivationFunctionType.Exp, mybir.ActivationFunctionType.Ln}
        tables = [
            (name, s if name == "natural_log_exp_and_others" else (s - shared))
            for name, s in tables
        ]
        _bass_rust.insert_act_table_loads(self, tables)


def _chunks():
    """(offset, width) pairs covering [0, NP), width % 128 == 0."""
    out = []
    off = 0
    while off < NP:
        w = min(CH, NP - off)
        out.append((off, w))
        off += w
    return out


def _spans():
    """Group chunks into spans of up to SPAN chunks."""
    ch = _chunks()
    spans = []
    for i in range(0, len(ch), SPAN):
        group = ch[i : i + SPAN]
        s0 = group[0][0]
        sp = sum(w for _, w in group)
        spans.append((s0, sp, group))
    return spans


def _qsplit(sp):
    """Split sp into NQ (offset, width) parts, each width % 128 == 0.

    Queue 0 runs the transposed gather, whose Q7 descriptor generation is
    ~20% slower per index than the plain mode on queues 1-3 - give it a
    correspondingly smaller share so all four Q7 pairs finish together.
    """
    n128 = sp // 128
    t = max(1, (n128 * 3) // 16)
    parts = [(0, t * 128)]
    rest = n128 - t
    off = t * 128
    for q in range(1, NQ):
        w = (rest // (NQ - 1) + (1 if (q - 1) < rest % (NQ - 1) else 0)) * 128
        if w:
            parts.append((off, w))
        off += w
    return parts


def _build():
    if "nc" in _CACHE:
        return _CACHE["nc"]
    nc = _Bacc(num_swdge_queues=NQ)

    x_d = nc.declare_dram_parameter("x", [N_ATOMS, F], f16, isOutput=False)
    rbf_d = nc.declare_dram_parameter("rbf_t", [R, NP], f16, isOutput=False)
    idx_d = nc.declare_dram_parameter("idx", [128, NP // 16], i16, isOutput=False)
    w1_d = nc.declare_dram_parameter("w1", [R, F], f16, isOutput=False)
    w2_d = nc.declare_dram_parameter("w2", [F, F], f16, isOutput=False)
    b1_d = nc.declare_dram_parameter("b1", [F, 1], f32, isOutput=False)
    b2_d = nc.declare_dram_parameter("b2", [F, 1], f32, isOutput=False)
    id_d = nc.declare_dram_parameter("ident", [F, F], f16, isOutput=False)
    out_d = nc.declare_dram_parameter("out", [F, NA], f32, isOutput=True)

    spans = _spans()
    max_sp = max(sp for _, sp, _ in spans)

    with tile.TileContext(nc) as tc, ExitStack() as ctx:
        consts = ctx.enter_context(tc.tile_pool(name="consts", bufs=1))
        # Span inputs (rbf slab + gathered x) prefetch up to 4 spans
        # ahead; span-local scratch only needs 2-deep rotation.
        inpool = ctx.enter_context(tc.tile_pool(name="inpool", bufs=4))
        scpool = ctx.enter_context(tc.tile_pool(name="scpool", bufs=3))
        # PSUM budget (8 banks of 512 f32): pp [128,CH] f32 = 2 banks
        # x3 bufs = 6 (mm1 writes it, exp reads it, then mm2 reuses it -
        # exp -> mm2 is a true dependency anyway, so the reuse costs no
        # ordering); xt (PE-transposed x_j, f16) [128,CH] = 1 bank x2 = 2.
        ppool = ctx.enter_context(tc.tile_pool(name="pp", bufs=3, space="PSUM"))
        xtpool = ctx.enter_context(tc.tile_pool(name="xt", bufs=2, space="PSUM"))

        # consts arrive via the ACT HWDGE queue: the sync queue then only
        # carries the latency-critical per-span rbft loads and output
        # stores, so span 0's rbft is never stuck behind the idx table.
        idxs = consts.tile([128, NP // 16], i16)
        sp0 = _spans()[0][1]
        nc.scalar.dma_start(
            out=idxs[:, : sp0 // 16], in_=idx_d[:, : sp0 // 16]
        )
        w1s = consts.tile([R, F], f16)
        nc.scalar.dma_start(out=w1s, in_=w1_d[:])
        w2s = consts.tile([F, F], f16)
        nc.scalar.dma_start(out=w2s, in_=w2_d[:])
        b1s = consts.tile([F, 1], f32)
        nc.scalar.dma_start(out=b1s, in_=b1_d[:])
        b2s = consts.tile([F, 1], f32)
        nc.scalar.dma_start(out=b2s, in_=b2_d[:])
        ids = consts.tile([F, F], f16)
        nc.scalar.dma_start(out=ids, in_=id_d[:])
        # DVE-local copy of b2: the STT multiply runs on DVE, so reading a
        # DVE-written tile adds no cross-engine sync wait.
        b2v = consts.tile([F, 1], f32)
        nc.vector.tensor_copy(out=b2v, in_=b2s)
        # (idx slice for span 0 was issued first above, so the first
        # gathers only wait on 64KB; the rest of the table follows here.)
        nc.scalar.dma_start(
            out=idxs[:, sp0 // 16 :], in_=idx_d[:, sp0 // 16 :]
        )
        outst = consts.tile([F, NA], f32)

        for s0, sp, group in spans:
            atom0 = s0 // NB
            natoms = sp // NB

            rbft = inpool.tile([R, max_sp], f16, tag="rbft")
            nc.sync.dma_start(out=rbft[:, :sp], in_=rbf_d[:, s0 : s0 + sp])

            xjg = inpool.tile([128, max_sp], f16, tag="xjg")
            xjt = inpool.tile([128, max_sp], f16, tag="xjt")
            xv = xjg.rearrange("p (g e) -> p g e", e=F)
            # Hybrid 4-way gather, one SWDGE queue per Q7 core pair:
            #  - queue 0 runs the TRANSPOSED gather (sole user of the
            #    SDMA xbar - concurrent transposed streams corrupt each
            #    other) and lands feature-major directly in xjt[:, :t].
            #  - queues 1-3 gather non-transposed into xjg; those groups
            #    are PE-transposed into xjt later.
            qparts = _qsplit(sp)
            t0w = qparts[0][1]
            nc.gpsimd.dma_gather(
                xjt[:, :t0w].rearrange("p (o n) -> p o n", o=1),
                x_d[:],
                idxs[:, s0 // 16 : (s0 + t0w) // 16],
                t0w,
                t0w,
                F,
                transpose=True,
                # single_packet=True crashes the SDMA engine above ~512
                # indices (NRT_EXEC_UNIT_UNRECOVERABLE) - packets are
                # capped at 64 descriptors.
                single_packet=False,
                queue_num=0,
            )
            for q, (qoff, qw) in enumerate(qparts[1:], start=1):
                nc.gpsimd.dma_gather(
                    xv[:, (qoff - t0w) // F : (qoff - t0w + qw) // F, :],
                    x_d[:],
                    idxs[:, (s0 + qoff) // 16 : (s0 + qoff + qw) // 16],
                    qw,
                    qw,
                    F,
                    transpose=False,
                    single_packet=False,
                    queue_num=q,
                )

            prod = scpool.tile([128, max_sp], f16, tag="prod")
            es = scpool.tile([128, max_sp], f16, tag="es")
            hsp = scpool.tile([128, max_sp], f16, tag="hsp")

            # Chunk loop, software-pipelined by one stage: mm2/STT of
            # chunk c-1 issue after mm1/transposes of chunk c, so the PE
            # never head-of-line blocks on ACT's ln while a ready mm1
            # waits behind mm2.
            def stage1(ci, off, w):
                co = off - s0
                pp = ppool.tile([128, CH], f32)
                for o in range(0, w, 512):
                    n = min(512, w - o)
                    nc.tensor.matmul(
                        pp[:, o : o + n],
                        w1s[:],
                        rbft[:, co + o : co + o + n],
                        start=True,
                        stop=True,
                    )
                # e = exp(h_pre + b1), then h = softplus = ln(e + 1); both
                # resolve to the natural_log_exp_and_others table -> no
                # table switches.
                nc.scalar.activation(
                    out=es[:, co : co + w],
                    in_=pp[:, :w],
                    func=mybir.ActivationFunctionType.Exp,
                    bias=b1s[:, 0:1],
                    scale=1.0,
                )
                nc.scalar.activation(
                    out=hsp[:, co : co + w],
                    in_=es[:, co : co + w],
                    func=mybir.ActivationFunctionType.Ln,
                    bias=1.0,
                    scale=1.0,
                )
                # xjt[f, p] per 128-pair group via PE transpose
                # (SBUF->PSUM), for the groups queue 0 didn't already land
                # feature-major.
                tco = max(co, t0w)
                if tco < co + w:
                    xt = xtpool.tile([128, CH], f16)
                    for g in range(tco - co, w, F):
                        nc.tensor.transpose(
                            xt[:, g : g + F],
                            xv[:, (co + g - t0w) // F, :],
                            ids[:],
                        )
                    # DVE reads only ONE non-scalar operand from PSUM; stage
                    # the transposed x_j in SBUF first. Alternate the copy
                    # between DVE and ACT to balance engine load.
                    if ci % 2 == 0:
                        nc.vector.tensor_copy(
                            out=xjt[:, tco : co + w],
                            in_=xt[:, tco - co : w],
                        )
                    else:
                        nc.scalar.activation(
                            out=xjt[:, tco : co + w],
                            in_=xt[:, tco - co : w],
                            func=mybir.ActivationFunctionType.Copy,
                            bias=0.0,
                            scale=1.0,
                        )
                return pp

            def stage2(off, w, pp):
                co = off - s0
                for o in range(0, w, 512):
                    n = min(512, w - o)
                    nc.tensor.matmul(
                        pp[:, o : o + n],
                        w2s[:],
                        hsp[:, co + o : co + o + n],
                        start=True,
                        stop=True,
                    )
                # prod = (W + b2) * xjt  (fused bias + multiply, PSUM read)
                nc.vector.scalar_tensor_tensor(
                    out=prod[:, co : co + w],
                    in0=pp[:, :w],
                    scalar=b2v[:, 0:1],
                    in1=xjt[:, co : co + w],
                    op0=mybir.AluOpType.add,
                    op1=mybir.AluOpType.mult,
                )

            def do_reduce(co, wid):
                # neighbor reduction straight into the f32 output staging
                # tile (tensor_reduce is hardwired PERF_ONE, so an f16
                # output would not run any faster). 2-chunk granularity
                # halves the per-instruction and semaphore overhead.
                nc.vector.tensor_reduce(
                    out=outst[:, (s0 + co) // NB : (s0 + co + wid) // NB],
                    in_=prod[:, co : co + wid].rearrange(
                        "p (a n) -> p a n", n=NB
                    ),
                    axis=mybir.AxisListType.X,
                    op=mybir.AluOpType.add,
                )

            pending = None
            red0 = 0
            for ci, (off, w) in enumerate(group):
                pp = stage1(ci, off, w)
                if pending is not None:
                    stage2(*pending)
                    if ci % 2 == 0:
                        # pending (odd ci-1) completed a 2-chunk pair
                        co_end = pending[0] - s0 + pending[1]
                        do_reduce(red0, co_end - red0)
                        red0 = co_end
                pending = (off, w, pp)
            stage2(*pending)
            do_reduce(red0, sp - red0)

            # stream this span's output slab out now (drain overlap)
            nc.sync.dma_start(
                out=out_d[:, atom0 : atom0 + natoms],
                in_=outst[:, atom0 : atom0 + natoms],
            )



    # Bacc.finalize() runs the sync-wait legalization (each TRN2 instruction
    # carries at most one wait; extras are split into event-semaphore insts).
    nc.finalize()
    _CACHE["nc"] = nc
    return nc


def _prep_core_inputs(x16, rbf, neighbors, w1_16, w2_16, b1c, b2c, ident, c):
    a0, a1 = c * NA, (c + 1) * NA
    rbf_t = np.ascontiguousarray(rbf[a0:a1].reshape(NP, R).T.astype(np.float16))
    nb = np.ascontiguousarray(neighbors[a0:a1]).reshape(NP).astype(np.int16)
    # dma_gather index layout: element i lives at [i % 16, i // 16],
    # 16-partition block replicated 8x down the partition dim.
    idx16 = np.ascontiguousarray(nb.reshape(NP // 16, 16).T)
    idx = np.tile(idx16, (8, 1))
    return {
        "x": x16,
        "rbf_t": rbf_t,
        "idx": np.ascontiguousarray(idx),
        "w1": w1_16,
        "w2": w2_16,
        "b1": b1c,
        "b2": b2c,
        "ident": ident,
    }


def kernel(x, rbf, neighbors, w1, b1, w2, b2):
    from concourse.bass_utils import run_bass_kernel_spmd

    x = np.asarray(x)
    rbf = np.asarray(rbf)
    neighbors = np.asarray(neighbors)
    w1 = np.asarray(w1)
    b1 = np.asarray(b1)
    w2 = np.asarray(w2)
    b2 = np.asarray(b2)

    nc = _build()

    x16 = x.astype(np.float16)
    w1_16 = np.ascontiguousarray(w1.astype(np.float16))
    w2_16 = w2.astype(np.float16)
    b1c = np.ascontiguousarray(b1.reshape(F, 1).astype(np.float32))
    b2c = np.ascontiguousarray(b2.reshape(F, 1).astype(np.float32))
    ident = np.eye(F, dtype=np.float16)

    in_maps = [
        _prep_core_inputs(x16, rbf, neighbors, w1_16, w2_16, b1c, b2c, ident, c)
        for c in range(NCORES)
    ]

    res = run_bass_kernel_spmd(
        nc,
        in_maps,
        core_ids=list(range(NCORES)),
        trace=bool(int(os.environ.get("CFCONV_TRACE", "0"))),
    )
    _CACHE["last_result"] = res

    out = np.concatenate(
        [res.results[c]["out"].T for c in range(NCORES)], axis=0
    )
    return np.ascontiguousarray(out.astype(np.float32))


# revision 43
# speedup vs baseline: 1.1064x; 1.0084x over previous
"""CFConv (SchNet continuous-filter convolution) on 8 TRN2 NeuronCores.

    h   = softplus(rbf @ w1 + b1)        # [N, NB, F]
    W   = h @ w2 + b2                    # [N, NB, F]
    out = sum_n x[neighbors] * W         # [N, F]

Sharding: atoms (dim 0) split 8 ways; x + filter weights replicated.
No collectives needed - each core gathers its neighbors from its own
full copy of x in DRAM.

Per-core dataflow (feature-major [f, pair] layout for the filter math):
  SWDGE:      xjg[p, g, e] = x[nbr[g*128+p], e]   (dma_gather
              transpose=False, split across all 4 SWDGE queues so the
              four Q7 core pairs generate descriptors concurrently;
              transpose=True mode is single-queue-only: concurrent
              transposed streams corrupt each other in the shared xbar)
  mm1 (PE):   h_pre[f, p] = w1[r, f].T @ rbf_t[r, p]      -> PSUM f32
  ACT:        e = exp(h_pre + b1)  (b1 = per-partition bias AP)
  ACT:        h = softplus = ln(e + 1)  (exp+ln pinned to the one
              act-func table containing both -> no table reloads)
  mm2 (PE):   W[f, p] = w2[g, f].T @ h[g, p]              -> PSUM f32
  PE:         xjt[f, p] = transpose(xjg group)            -> PSUM f32
  DVE STT:    prod = (W + b2) * xjt   (both operands read from PSUM)
  DVE:        out[f, a] = tensor_reduce over the 32-neighbor axis
Output is [128 f, 2500 atoms] f32 per core; host transposes + concats.
"""

import os

import numpy as np

import concourse.bass as bass
import concourse.bacc as bacc
import concourse.mybir as mybir
import concourse.tile as tile
from contextlib import ExitStack

N_ATOMS = 20000
NB = 32
F = 128
R = 64
NCORES = 8
NA = N_ATOMS // NCORES          # atoms per core     = 2500
NP = NA * NB                    # pairs per core     = 80000
CH = 1024                       # pairs per chunk
SPAN = 4                        # chunks per span (gather/reduce granularity)
NQ = 4                          # SWDGE queues (Q7 core pairs) for gathers

f16 = mybir.dt.float16
f32 = mybir.dt.float32
i16 = mybir.dt.int16

_CACHE = {}


class _Bacc(bacc.Bacc):
    """Bacc with exp+ln pinned to the one act-func table containing both.

    The stock placement pass picks the first act_info.json set containing
    each activation function: Exp -> "exp_and_others", Ln -> "natural_log",
    which alternates ACT_TABLE_LOADs (~1.5us each) on every exp/ln switch.
    Removing Exp/Ln from every other set (list order preserved, so the
    positional act_func_set_id stays valid) pins both to
    "natural_log_exp_and_others" -> one load for the whole kernel.
    """

    def insert_act_table_loads(self):
        from concourse.hw_specs import get_activation_tables
        from concourse.bacc import _bass_rust

        has_activation = any(
            isinstance(i, mybir.InstActivation)
            for b in self.main_func.blocks
            for i in b.instructions
        )
        if not has_activation:
            return
        tables = list(get_activation_tables(self.m.arch).items())
        shared = {mybir.ActivationFunctionType.Exp, mybir.ActivationFunctionType.Ln}
        tables = [
            (name, s if name == "natural_log_exp_and_others" else (s - shared))
            for name, s in tables
        ]
        _bass_rust.insert_act_table_loads(self, tables)


def _chunks():
    """(offset, width) pairs covering [0, NP), width % 128 == 0."""
    out = []
    off = 0
    while off < NP:
        w = min(CH, NP - off)
        out.append((off, w))
        off += w
    return out


def _spans():
    """Group chunks into spans of up to SPAN chunks."""
    ch = _chunks()
    spans = []
    for i in range(0, len(ch), SPAN):
        group = ch[i : i + SPAN]
        s0 = group[0][0]
        sp = sum(w for _, w in group)
        spans.append((s0, sp, group))
    return spans


def _qsplit(sp):
    """Split sp into NQ (offset, width) parts, each width % 128 == 0.

    Queue 0 runs the transposed gather, whose Q7 descriptor generation is
    ~20% slower per index than the plain mode on queues 1-3 - give it a
    correspondingly smaller share so all four Q7 pairs finish together.
    """
    n128 = sp // 128
    t = max(1, (n128 * 3) // 16)
    parts = [(0, t * 128)]
    rest = n128 - t
    off = t * 128
    for q in range(1, NQ):
        w = (rest // (NQ - 1) + (1 if (q - 1) < rest % (NQ - 1) else 0)) * 128
        if w:
            parts.append((off, w))
        off += w
    return parts


def _build():
    if "nc" in _CACHE:
        return _CACHE["nc"]
    nc = _Bacc(num_swdge_queues=NQ, dynamic_dma_scratch_size=32768)

    x_d = nc.declare_dram_parameter("x", [N_ATOMS, F], f16, isOutput=False)
    rbf_d = nc.declare_dram_parameter("rbf_t", [R, NP], f16, isOutput=False)
    idx_d = nc.declare_dram_parameter("idx", [128, NP // 16], i16, isOutput=False)
    w1_d = nc.declare_dram_parameter("w1", [R, F], f16, isOutput=False)
    w2_d = nc.declare_dram_parameter("w2", [F, F], f16, isOutput=False)
    b1_d = nc.declare_dram_parameter("b1", [F, 1], f32, isOutput=False)
    b2_d = nc.declare_dram_parameter("b2", [F, 1], f32, isOutput=False)
    id_d = nc.declare_dram_parameter("ident", [F, F], f16, isOutput=False)
    out_d = nc.declare_dram_parameter("out", [F, NA], f32, isOutput=True)

    spans = _spans()
    max_sp = max(sp for _, sp, _ in spans)

    with tile.TileContext(nc) as tc, ExitStack() as ctx:
        consts = ctx.enter_context(tc.tile_pool(name="consts", bufs=1))
        # Span inputs (rbf slab + gathered x) prefetch up to 4 spans
        # ahead; span-local scratch only needs 2-deep rotation.
        inpool = ctx.enter_context(tc.tile_pool(name="inpool", bufs=4))
        scpool = ctx.enter_context(tc.tile_pool(name="scpool", bufs=3))
        # PSUM budget (8 banks of 512 f32): pp [128,CH] f32 = 2 banks
        # x3 bufs = 6 (mm1 writes it, exp reads it, then mm2 reuses it -
        # exp -> mm2 is a true dependency anyway, so the reuse costs no
        # ordering); xt (PE-transposed x_j, f16) [128,CH] = 1 bank x2 = 2.
        ppool = ctx.enter_context(tc.tile_pool(name="pp", bufs=3, space="PSUM"))
        xtpool = ctx.enter_context(tc.tile_pool(name="xt", bufs=2, space="PSUM"))

        # consts arrive via the ACT HWDGE queue: the sync queue then only
        # carries the latency-critical per-span rbft loads and output
        # stores, so span 0's rbft is never stuck behind the idx table.
        idxs = consts.tile([128, NP // 16], i16)
        sp0 = _spans()[0][1]
        nc.scalar.dma_start(
            out=idxs[:, : sp0 // 16], in_=idx_d[:, : sp0 // 16]
        )
        w1s = consts.tile([R, F], f16)
        nc.scalar.dma_start(out=w1s, in_=w1_d[:])
        w2s = consts.tile([F, F], f16)
        nc.scalar.dma_start(out=w2s, in_=w2_d[:])
        b1s = consts.tile([F, 1], f32)
        nc.scalar.dma_start(out=b1s, in_=b1_d[:])
        b2s = consts.tile([F, 1], f32)
        nc.scalar.dma_start(out=b2s, in_=b2_d[:])
        ids = consts.tile([F, F], f16)
        nc.scalar.dma_start(out=ids, in_=id_d[:])
        # DVE-local copy of b2: the STT multiply runs on DVE, so reading a
        # DVE-written tile adds no cross-engine sync wait.
        b2v = consts.tile([F, 1], f32)
        nc.vector.tensor_copy(out=b2v, in_=b2s)
        # (idx slice for span 0 was issued first above, so the first
        # gathers only wait on 64KB; the rest of the table follows here.)
        nc.scalar.dma_start(
            out=idxs[:, sp0 // 16 :], in_=idx_d[:, sp0 // 16 :]
        )
        outst = consts.tile([F, NA], f32)

        for s0, sp, group in spans:
            atom0 = s0 // NB
            natoms = sp // NB

            rbft = inpool.tile([R, max_sp], f16, tag="rbft")
            nc.sync.dma_start(out=rbft[:, :sp], in_=rbf_d[:, s0 : s0 + sp])

            xjg = inpool.tile([128, max_sp], f16, tag="xjg")
            xjt = inpool.tile([128, max_sp], f16, tag="xjt")
            xv = xjg.rearrange("p (g e) -> p g e", e=F)
            # Hybrid 4-way gather, one SWDGE queue per Q7 core pair:
            #  - queue 0 runs the TRANSPOSED gather (sole user of the
            #    SDMA xbar - concurrent transposed streams corrupt each
            #    other) and lands feature-major directly in xjt[:, :t].
            #  - queues 1-3 gather non-transposed into xjg; those groups
            #    are PE-transposed into xjt later.
            qparts = _qsplit(sp)
            t0w = qparts[0][1]
            nc.gpsimd.dma_gather(
                xjt[:, :t0w].rearrange("p (o n) -> p o n", o=1),
                x_d[:],
                idxs[:, s0 // 16 : (s0 + t0w) // 16],
                t0w,
                t0w,
                F,
                transpose=True,
                # single_packet=True crashes the SDMA engine above ~512
                # indices (NRT_EXEC_UNIT_UNRECOVERABLE) - packets are
                # capped at 64 descriptors.
                single_packet=False,
                queue_num=0,
            )
            for q, (qoff, qw) in enumerate(qparts[1:], start=1):
                nc.gpsimd.dma_gather(
                    xv[:, (qoff - t0w) // F : (qoff - t0w + qw) // F, :],
                    x_d[:],
                    idxs[:, (s0 + qoff) // 16 : (s0 + qoff + qw) // 16],
                    qw,
                    qw,
                    F,
                    transpose=False,
                    single_packet=False,
                    queue_num=q,
                )

            prod = scpool.tile([128, max_sp], f16, tag="prod")
            es = scpool.tile([128, max_sp], f16, tag="es")
            hsp = scpool.tile([128, max_sp], f16, tag="hsp")

            # Chunk loop, software-pipelined by one stage: mm2/STT of
            # chunk c-1 issue after mm1/transposes of chunk c, so the PE
            # never head-of-line blocks on ACT's ln while a ready mm1
            # waits behind mm2.
            def stage1(ci, off, w):
                co = off - s0
                pp = ppool.tile([128, CH], f32)
                for o in range(0, w, 512):
                    n = min(512, w - o)
                    nc.tensor.matmul(
                        pp[:, o : o + n],
                        w1s[:],
                        rbft[:, co + o : co + o + n],
                        start=True,
                        stop=True,
                    )
                # e = exp(h_pre + b1), then h = softplus = ln(e + 1); both
                # resolve to the natural_log_exp_and_others table -> no
                # table switches.
                nc.scalar.activation(
                    out=es[:, co : co + w],
                    in_=pp[:, :w],
                    func=mybir.ActivationFunctionType.Exp,
                    bias=b1s[:, 0:1],
                    scale=1.0,
                )
                nc.scalar.activation(
                    out=hsp[:, co : co + w],
                    in_=es[:, co : co + w],
                    func=mybir.ActivationFunctionType.Ln,
                    bias=1.0,
                    scale=1.0,
                )
                # xjt[f, p] per 128-pair group via PE transpose
                # (SBUF->PSUM), for the groups queue 0 didn't already land
                # feature-major.
                tco = max(co, t0w)
                if tco < co + w:
                    xt = xtpool.tile([128, CH], f16)
                    for g in range(tco - co, w, F):
                        nc.tensor.transpose(
                            xt[:, g : g + F],
                            xv[:, (co + g - t0w) // F, :],
                            ids[:],
                        )
                    # DVE reads only ONE non-scalar operand from PSUM; stage
                    # the transposed x_j in SBUF first. Alternate the copy
                    # between DVE and ACT to balance engine load.
                    if ci % 2 == 0:
                        nc.vector.tensor_copy(
                            out=xjt[:, tco : co + w],
                            in_=xt[:, tco - co : w],
                        )
                    else:
                        nc.scalar.activation(
                            out=xjt[:, tco : co + w],
                            in_=xt[:, tco - co : w],
                            func=mybir.ActivationFunctionType.Copy,
                            bias=0.0,
                            scale=1.0,
                        )
                return pp

            def stage2(off, w, pp):
                co = off - s0
                for o in range(0, w, 512):
                    n = min(512, w - o)
                    nc.tensor.matmul(
                        pp[:, o : o + n],
                        w2s[:],
                        hsp[:, co + o : co + o + n],
                        start=True,
                        stop=True,
                    )
                # prod = (W + b2) * xjt  (fused bias + multiply, PSUM read)
                nc.vector.scalar_tensor_tensor(
                    out=prod[:, co : co + w],
                    in0=pp[:, :w],
                    scalar=b2v[:, 0:1],
                    in1=xjt[:, co : co + w],
                    op0=mybir.AluOpType.add,
                    op1=mybir.AluOpType.mult,
                )

            def do_reduce(co, wid):
                # neighbor reduction straight into the f32 output staging
                # tile (tensor_reduce is hardwired PERF_ONE, so an f16
                # output would not run any faster). 2-chunk granularity
                # halves the per-instruction and semaphore overhead.
                nc.vector.tensor_reduce(
                    out=outst[:, (s0 + co) // NB : (s0 + co + wid) // NB],
                    in_=prod[:, co : co + wid].rearrange(
                        "p (a n) -> p a n", n=NB
                    ),
                    axis=mybir.AxisListType.X,
                    op=mybir.AluOpType.add,
                )

            pending = None
            red0 = 0
            for ci, (off, w) in enumerate(group):
                pp = stage1(ci, off, w)
                if pending is not None:
                    stage2(*pending)
                    if ci % 2 == 0:
                        # pending (odd ci-1) completed a 2-chunk pair
                        co_end = pending[0] - s0 + pending[1]
                        do_reduce(red0, co_end - red0)
                        red0 = co_end
                pending = (off, w, pp)
            stage2(*pending)
            do_reduce(red0, sp - red0)

            # stream this span's output slab out now (drain overlap)
            nc.sync.dma_start(
                out=out_d[:, atom0 : atom0 + natoms],
                in_=outst[:, atom0 : atom0 + natoms],
            )



    # Bacc.finalize() runs the sync-wait legalization (each TRN2 instruction
    # carries at most one wait; extras are split into event-semaphore insts).
    nc.finalize()
    _CACHE["nc"] = nc
    return nc


def _prep_core_inputs(x16, rbf, neighbors, w1_16, w2_16, b1c, b2c, ident, c):
    a0, a1 = c * NA, (c + 1) * NA
    rbf_t = np.ascontiguousarray(rbf[a0:a1].reshape(NP, R).T.astype(np.float16))
    nb = np.ascontiguousarray(neighbors[a0:a1]).reshape(NP).astype(np.int16)
    # dma_gather index layout: element i lives at [i % 16, i // 16],
    # 16-partition block replicated 8x down the partition dim.
    idx16 = np.ascontiguousarray(nb.reshape(NP // 16, 16).T)
    idx = np.tile(idx16, (8, 1))
    return {
        "x": x16,
        "rbf_t": rbf_t,
        "idx": np.ascontiguousarray(idx),
        "w1": w1_16,
        "w2": w2_16,
        "b1": b1c,
        "b2": b2c,
        "ident": ident,
    }


def kernel(x, rbf, neighbors, w1, b1, w2, b2):
    from concourse.bass_utils import run_bass_kernel_spmd

    x = np.asarray(x)
    rbf = np.asarray(rbf)
    neighbors = np.asarray(neighbors)
    w1 = np.asarray(w1)
    b1 = np.asarray(b1)
    w2 = np.asarray(w2)
    b2 = np.asarray(b2)

    nc = _build()

    x16 = x.astype(np.float16)
    w1_16 = np.ascontiguousarray(w1.astype(np.float16))
    w2_16 = w2.astype(np.float16)
    b1c = np.ascontiguousarray(b1.reshape(F, 1).astype(np.float32))
    b2c = np.ascontiguousarray(b2.reshape(F, 1).astype(np.float32))
    ident = np.eye(F, dtype=np.float16)

    in_maps = [
        _prep_core_inputs(x16, rbf, neighbors, w1_16, w2_16, b1c, b2c, ident, c)
        for c in range(NCORES)
    ]

    res = run_bass_kernel_spmd(
        nc,
        in_maps,
        core_ids=list(range(NCORES)),
        trace=bool(int(os.environ.get("CFCONV_TRACE", "0"))),
    )
    _CACHE["last_result"] = res

    out = np.concatenate(
        [res.results[c]["out"].T for c in range(NCORES)], axis=0
    )
    return np.ascontiguousarray(out.astype(np.float32))


# revision 44
# speedup vs baseline: 1.1131x; 1.0060x over previous
"""CFConv (SchNet continuous-filter convolution) on 8 TRN2 NeuronCores.

    h   = softplus(rbf @ w1 + b1)        # [N, NB, F]
    W   = h @ w2 + b2                    # [N, NB, F]
    out = sum_n x[neighbors] * W         # [N, F]

Sharding: atoms (dim 0) split 8 ways; x + filter weights replicated.
No collectives needed - each core gathers its neighbors from its own
full copy of x in DRAM.

Per-core dataflow (feature-major [f, pair] layout for the filter math):
  SWDGE:      xjg[p, g, e] = x[nbr[g*128+p], e]   (dma_gather
              transpose=False, split across all 4 SWDGE queues so the
              four Q7 core pairs generate descriptors concurrently;
              transpose=True mode is single-queue-only: concurrent
              transposed streams corrupt each other in the shared xbar)
  mm1 (PE):   h_pre[f, p] = w1[r, f].T @ rbf_t[r, p]      -> PSUM f32
  ACT:        e = exp(h_pre + b1)  (b1 = per-partition bias AP)
  ACT:        h = softplus = ln(e + 1)  (exp+ln pinned to the one
              act-func table containing both -> no table reloads)
  mm2 (PE):   W[f, p] = w2[g, f].T @ h[g, p]              -> PSUM f32
  PE:         xjt[f, p] = transpose(xjg group)            -> PSUM f32
  DVE STT:    prod = (W + b2) * xjt   (both operands read from PSUM)
  DVE:        out[f, a] = tensor_reduce over the 32-neighbor axis
Output is [128 f, 2500 atoms] f32 per core; host transposes + concats.
"""

import os

import numpy as np

import concourse.bass as bass
import concourse.bacc as bacc
import concourse.mybir as mybir
import concourse.tile as tile
from contextlib import ExitStack

N_ATOMS = 20000
NB = 32
F = 128
R = 64
NCORES = 8
NA = N_ATOMS // NCORES          # atoms per core     = 2500
NP = NA * NB                    # pairs per core     = 80000
CH = 1024                       # pairs per chunk
SPAN = 4                        # chunks per span (gather/reduce granularity)
NQ = 4                          # SWDGE queues (Q7 core pairs) for gathers

f16 = mybir.dt.float16
f32 = mybir.dt.float32
i16 = mybir.dt.int16

_CACHE = {}


class _Bacc(bacc.Bacc):
    """Bacc with exp+ln pinned to the one act-func table containing both.

    The stock placement pass picks the first act_info.json set containing
    each activation function: Exp -> "exp_and_others", Ln -> "natural_log",
    which alternates ACT_TABLE_LOADs (~1.5us each) on every exp/ln switch.
    Removing Exp/Ln from every other set (list order preserved, so the
    positional act_func_set_id stays valid) pins both to
    "natural_log_exp_and_others" -> one load for the whole kernel.
    """

    def insert_act_table_loads(self):
        from concourse.hw_specs import get_activation_tables
        from concourse.bacc import _bass_rust

        has_activation = any(
            isinstance(i, mybir.InstActivation)
            for b in self.main_func.blocks
            for i in b.instructions
        )
        if not has_activation:
            return
        tables = list(get_activation_tables(self.m.arch).items())
        shared = {mybir.ActivationFunctionType.Exp, mybir.ActivationFunctionType.Ln}
        tables = [
            (name, s if name == "natural_log_exp_and_others" else (s - shared))
            for name, s in tables
        ]
        _bass_rust.insert_act_table_loads(self, tables)


def _chunks():
    """(offset, width) pairs covering [0, NP), width % 128 == 0."""
    out = []
    off = 0
    while off < NP:
        w = min(CH, NP - off)
        out.append((off, w))
        off += w
    return out


def _spans():
    """Group chunks into spans of up to SPAN chunks."""
    ch = _chunks()
    spans = []
    for i in range(0, len(ch), SPAN):
        group = ch[i : i + SPAN]
        s0 = group[0][0]
        sp = sum(w for _, w in group)
        spans.append((s0, sp, group))
    return spans


def _qsplit(sp):
    """Split sp into NQ (offset, width) parts, each width % 128 == 0.

    Queue 0 runs the transposed gather, whose Q7 descriptor generation is
    ~20% slower per index than the plain mode on queues 1-3 - give it a
    correspondingly smaller share so all four Q7 pairs finish together.
    """
    n128 = sp // 128
    t = max(1, (n128 * 3) // 16)
    parts = [(0, t * 128)]
    rest = n128 - t
    off = t * 128
    for q in range(1, NQ):
        w = (rest // (NQ - 1) + (1 if (q - 1) < rest % (NQ - 1) else 0)) * 128
        if w:
            parts.append((off, w))
        off += w
    return parts


def _build():
    if "nc" in _CACHE:
        return _CACHE["nc"]
    nc = _Bacc(num_swdge_queues=NQ)

    x_d = nc.declare_dram_parameter("x", [N_ATOMS, F], f16, isOutput=False)
    rbf_d = nc.declare_dram_parameter("rbf_t", [R, NP], f16, isOutput=False)
    idx_d = nc.declare_dram_parameter("idx", [128, NP // 16], i16, isOutput=False)
    w1_d = nc.declare_dram_parameter("w1", [R, F], f16, isOutput=False)
    w2_d = nc.declare_dram_parameter("w2", [F, F], f16, isOutput=False)
    b1_d = nc.declare_dram_parameter("b1", [F, 1], f32, isOutput=False)
    b2_d = nc.declare_dram_parameter("b2", [F, 1], f32, isOutput=False)
    id_d = nc.declare_dram_parameter("ident", [F, F], f16, isOutput=False)
    out_d = nc.declare_dram_parameter("out", [F, NA], f32, isOutput=True)

    spans = _spans()
    max_sp = max(sp for _, sp, _ in spans)

    with tile.TileContext(nc) as tc, ExitStack() as ctx:
        consts = ctx.enter_context(tc.tile_pool(name="consts", bufs=1))
        # Span inputs (rbf slab + gathered x) prefetch up to 4 spans
        # ahead; span-local scratch only needs 2-deep rotation.
        inpool = ctx.enter_context(tc.tile_pool(name="inpool", bufs=4))
        scpool = ctx.enter_context(tc.tile_pool(name="scpool", bufs=3))
        # PSUM budget (8 banks of 512 f32): pp [128,CH] f32 = 2 banks
        # x3 bufs = 6 (mm1 writes it, exp reads it, then mm2 reuses it -
        # exp -> mm2 is a true dependency anyway, so the reuse costs no
        # ordering); xt (PE-transposed x_j, f16) [128,CH] = 1 bank x2 = 2.
        ppool = ctx.enter_context(tc.tile_pool(name="pp", bufs=3, space="PSUM"))
        xtpool = ctx.enter_context(tc.tile_pool(name="xt", bufs=2, space="PSUM"))

        # consts arrive via the ACT HWDGE queue: the sync queue then only
        # carries the latency-critical per-span rbft loads and output
        # stores, so span 0's rbft is never stuck behind the idx table.
        idxs = consts.tile([128, NP // 16], i16)
        sp0 = _spans()[0][1]
        nc.scalar.dma_start(
            out=idxs[:, : sp0 // 16], in_=idx_d[:, : sp0 // 16]
        )
        w1s = consts.tile([R, F], f16)
        nc.scalar.dma_start(out=w1s, in_=w1_d[:])
        w2s = consts.tile([F, F], f16)
        nc.scalar.dma_start(out=w2s, in_=w2_d[:])
        b1s = consts.tile([F, 1], f32)
        nc.scalar.dma_start(out=b1s, in_=b1_d[:])
        b2s = consts.tile([F, 1], f32)
        nc.scalar.dma_start(out=b2s, in_=b2_d[:])
        ids = consts.tile([F, F], f16)
        nc.scalar.dma_start(out=ids, in_=id_d[:])
        # DVE-local copy of b2: the STT multiply runs on DVE, so reading a
        # DVE-written tile adds no cross-engine sync wait.
        b2v = consts.tile([F, 1], f32)
        nc.vector.tensor_copy(out=b2v, in_=b2s)
        # (idx slice for span 0 was issued first above, so the first
        # gathers only wait on 64KB; the rest of the table follows here.)
        nc.scalar.dma_start(
            out=idxs[:, sp0 // 16 :], in_=idx_d[:, sp0 // 16 :]
        )
        outst = consts.tile([F, NA], f32)

        for s0, sp, group in spans:
            atom0 = s0 // NB
            natoms = sp // NB

            rbft = inpool.tile([R, max_sp], f16, tag="rbft")
            nc.sync.dma_start(out=rbft[:, :sp], in_=rbf_d[:, s0 : s0 + sp])

            xjg = inpool.tile([128, max_sp], f16, tag="xjg")
            xjt = inpool.tile([128, max_sp], f16, tag="xjt")
            xv = xjg.rearrange("p (g e) -> p g e", e=F)
            # Hybrid 4-way gather, one SWDGE queue per Q7 core pair:
            #  - queue 0 runs the TRANSPOSED gather (sole user of the
            #    SDMA xbar - concurrent transposed streams corrupt each
            #    other) and lands feature-major directly in xjt[:, :t].
            #  - queues 1-3 gather non-transposed into xjg; those groups
            #    are PE-transposed into xjt later.
            qparts = _qsplit(sp)
            t0w = qparts[0][1]
            nc.gpsimd.dma_gather(
                xjt[:, :t0w].rearrange("p (o n) -> p o n", o=1),
                x_d[:],
                idxs[:, s0 // 16 : (s0 + t0w) // 16],
                t0w,
                t0w,
                F,
                transpose=True,
                # single_packet=True crashes the SDMA engine above ~512
                # indices (NRT_EXEC_UNIT_UNRECOVERABLE) - packets are
                # capped at 64 descriptors.
                single_packet=False,
                queue_num=0,
            )
            for q, (qoff, qw) in enumerate(qparts[1:], start=1):
                nc.gpsimd.dma_gather(
                    xv[:, (qoff - t0w) // F : (qoff - t0w + qw) // F, :],
                    x_d[:],
                    idxs[:, (s0 + qoff) // 16 : (s0 + qoff + qw) // 16],
                    qw,
                    qw,
                    F,
                    transpose=False,
                    single_packet=False,
                    queue_num=q,
                )

            prod = scpool.tile([128, max_sp], f16, tag="prod")
            es = scpool.tile([128, max_sp], f16, tag="es")
            hsp = scpool.tile([128, max_sp], f16, tag="hsp")

            # Chunk loop, software-pipelined by one stage: mm2/STT of
            # chunk c-1 issue after mm1/transposes of chunk c, so the PE
            # never head-of-line blocks on ACT's ln while a ready mm1
            # waits behind mm2.
            def stage1(ci, off, w):
                co = off - s0
                pp = ppool.tile([128, CH], f32)
                for o in range(0, w, 512):
                    n = min(512, w - o)
                    nc.tensor.matmul(
                        pp[:, o : o + n],
                        w1s[:],
                        rbft[:, co + o : co + o + n],
                        start=True,
                        stop=True,
                    )
                # e = exp(h_pre + b1), then h = softplus = ln(e + 1); both
                # resolve to the natural_log_exp_and_others table -> no
                # table switches.
                nc.scalar.activation(
                    out=es[:, co : co + w],
                    in_=pp[:, :w],
                    func=mybir.ActivationFunctionType.Exp,
                    bias=b1s[:, 0:1],
                    scale=1.0,
                )
                nc.scalar.activation(
                    out=hsp[:, co : co + w],
                    in_=es[:, co : co + w],
                    func=mybir.ActivationFunctionType.Ln,
                    bias=1.0,
                    scale=1.0,
                )
                # xjt[f, p] per 128-pair group via PE transpose
                # (SBUF->PSUM), for the groups queue 0 didn't already land
                # feature-major.
                tco = max(co, t0w)
                if tco < co + w:
                    xt = xtpool.tile([128, CH], f16)
                    for g in range(tco - co, w, F):
                        nc.tensor.transpose(
                            xt[:, g : g + F],
                            xv[:, (co + g - t0w) // F, :],
                            ids[:],
                        )
                    # DVE reads only ONE non-scalar operand from PSUM; stage
                    # the transposed x_j in SBUF first. Alternate the copy
                    # between DVE and ACT to balance engine load.
                    if ci % 2 == 0:
                        nc.vector.tensor_copy(
                            out=xjt[:, tco : co + w],
                            in_=xt[:, tco - co : w],
                        )
                    else:
                        nc.scalar.activation(
                            out=xjt[:, tco : co + w],
                            in_=xt[:, tco - co : w],
                            func=mybir.ActivationFunctionType.Copy,
                            bias=0.0,
                            scale=1.0,
                        )
                return pp

            def stage2(off, w, pp):
                co = off - s0
                for o in range(0, w, 512):
                    n = min(512, w - o)
                    nc.tensor.matmul(
                        pp[:, o : o + n],
                        w2s[:],
                        hsp[:, co + o : co + o + n],
                        start=True,
                        stop=True,
                    )
                # prod = (W + b2) * xjt  (fused bias + multiply, PSUM read)
                nc.vector.scalar_tensor_tensor(
                    out=prod[:, co : co + w],
                    in0=pp[:, :w],
                    scalar=b2v[:, 0:1],
                    in1=xjt[:, co : co + w],
                    op0=mybir.AluOpType.add,
                    op1=mybir.AluOpType.mult,
                )

            def do_reduce(co, wid):
                # neighbor reduction straight into the f32 output staging
                # tile (tensor_reduce is hardwired PERF_ONE, so an f16
                # output would not run any faster). 2-chunk granularity
                # halves the per-instruction and semaphore overhead.
                nc.vector.tensor_reduce(
                    out=outst[:, (s0 + co) // NB : (s0 + co + wid) // NB],
                    in_=prod[:, co : co + wid].rearrange(
                        "p (a n) -> p a n", n=NB
                    ),
                    axis=mybir.AxisListType.X,
                    op=mybir.AluOpType.add,
                )

            pending = None
            red0 = 0
            for ci, (off, w) in enumerate(group):
                pp = stage1(ci, off, w)
                if pending is not None:
                    stage2(*pending)
                    if ci % 2 == 0:
                        # pending (odd ci-1) completed a 2-chunk pair
                        co_end = pending[0] - s0 + pending[1]
                        do_reduce(red0, co_end - red0)
                        red0 = co_end
                pending = (off, w, pp)
            stage2(*pending)
            do_reduce(red0, sp - red0)

            # stream this span's output slab out now (drain overlap)
            nc.sync.dma_start(
                out=out_d[:, atom0 : atom0 + natoms],
                in_=outst[:, atom0 : atom0 + natoms],
            )



    # Bacc.finalize() runs the sync-wait legalization (each TRN2 instruction
    # carries at most one wait; extras are split into event-semaphore insts).
    nc.finalize()
    _CACHE["nc"] = nc
    return nc


def _prep_core_inputs(x16, rbf, neighbors, w1_16, w2_16, b1c, b2c, ident, c):
    a0, a1 = c * NA, (c + 1) * NA
    rbf_t = np.ascontiguousarray(rbf[a0:a1].reshape(NP, R).T.astype(np.float16))
    nb = np.ascontiguousarray(neighbors[a0:a1]).reshape(NP).astype(np.int16)
    # dma_gather index layout: element i lives at [i % 16, i // 16],
    # 16-partition block replicated 8x down the partition dim.
    idx16 = np.ascontiguousarray(nb.reshape(NP // 16, 16).T)
    idx = np.tile(idx16, (8, 1))
    return {
        "x": x16,
        "rbf_t": rbf_t,
        "idx": np.ascontiguousarray(idx),
        "w1": w1_16,
        "w2": w2_16,
        "b1": b1c,
        "b2": b2c,
        "ident": ident,
    }


def kernel(x, rbf, neighbors, w1, b1, w2, b2):
    from concourse.bass_utils import run_bass_kernel_spmd

    x = np.asarray(x)
    rbf = np.asarray(rbf)
    neighbors = np.asarray(neighbors)
    w1 = np.asarray(w1)
    b1 = np.asarray(b1)
    w2 = np.asarray(w2)
    b2 = np.asarray(b2)

    nc = _build()

    x16 = x.astype(np.float16)
    w1_16 = np.ascontiguousarray(w1.astype(np.float16))
    w2_16 = w2.astype(np.float16)
    b1c = np.ascontiguousarray(b1.reshape(F, 1).astype(np.float32))
    b2c = np.ascontiguousarray(b2.reshape(F, 1).astype(np.float32))
    ident = np.eye(F, dtype=np.float16)

    in_maps = [
        _prep_core_inputs(x16, rbf, neighbors, w1_16, w2_16, b1c, b2c, ident, c)
        for c in range(NCORES)
    ]

    res = run_bass_kernel_spmd(
        nc,
        in_maps,
        core_ids=list(range(NCORES)),
        trace=bool(int(os.environ.get("CFCONV_TRACE", "0"))),
    )
    _CACHE["last_result"] = res

    out = np.concatenate(
        [res.results[c]["out"].T for c in range(NCORES)], axis=0
    )
    return np.ascontiguousarray(out.astype(np.float32))
